# revision 27
# baseline (speedup 1.0000x reference)
"""Trainium2 Bass kernel for a GPT-style transformer block.

Reference computation (B=2, T=2048, D=1024, H=16 heads, causal):
    x = x + attn(LN1(x)) @ Wproj ;  x = x + relu(LN2(x) @ Wfc) @ Wmlp

Distribution over 8 NeuronCores (head-parallel front, token-parallel
tail), with everything kept in feature-major (transposed) layout:

  - LN1 stats: each core computes mean/std for its own 512-token block
    (ones-matmuls over x^T), AllGathers the [mu; sd] rows, and the LN
    rank-1 correction is folded into the QKV matmul as two extra
    contraction rows (lhsT = [ncolsum; wbias], rhs = [mu_row; sd_row]).
    The per-token 1/sd scale is one DVE tensor-tensor per output, with
    1/sd computed as exp(-ln(sd)) on ScalarE (no slow DVE reciprocal).
  - Causal attention per (batch, q-block) in transposed score layout
    S^T[k, q]; both heads' score matmuls are row-tiled into the PE
    concurrently and share one [128, 1024] PSUM tile so a single
    ScalarE exp serves both heads.  Softmax denominators fall out of a
    ones-augmented V column; y stays UNNORMALIZED and the denominator
    rows travel through the AllToAll (130 rows per rank) — the
    receiver does one batched reciprocal.
  - One bf16 AllToAll per batch re-shards y^T from head-parallel to
    token-parallel.  Each core owns 256 tokens of each batch.
  - Tail in transposed layout: attn-proj accumulates z^T directly (no
    PE transposes), the residual comes from x^T, and LN2 is folded
    into the fc matmul via relu(s*u) = s*relu(u) with the s scale
    pulled through the whole MLP (bias terms ride extra contraction
    rows).  The final output is written transposed [D, tok] and the
    host transposes while assembling the full output.

kernel(**inputs) takes the full unsharded inputs and returns the full
[2, 2048, 1024] output.
"""

import ml_dtypes
import numpy as np

import concourse.bacc as bacc
import concourse.tile as tile
import concourse.mybir as mybir
from concourse.bass_utils import run_bass_kernel_spmd

# Problem shape (hardcoded per the grading contract).
B, T, D = 2, 2048, 1024
H = 16
NC = 8                           # cores
TOK = B * T                      # 4096 tokens
BLK = 512                        # ph1 block size (tokens)
NB = TOK // BLK                  # 8 token blocks
DC = D // 128                    # 8 d-chunks
FC = 4 * D                       # 4096 mlp hidden
NFC = FC // 128                  # 32 fc slices
HB = T // NC                     # 256 tokens per core per batch
EPS = 1e-5

DBG = False

F32 = mybir.dt.float32
BF16 = mybir.dt.bfloat16
AL = mybir.AluOpType
AF = mybir.ActivationFunctionType


def build():
    nc = bacc.Bacc("TRN2", target_bir_lowering=False, debug=False,
                   num_devices=NC)

    io = {}

    def din(name, shape, dt=BF16):
        io[name] = nc.dram_tensor(name, shape, dt,
                                  kind="ExternalInput").ap()

    din("xT", [D, TOK])                 # x^T (shared)
    din("xo", [D, 2 * HB])              # own token cols b0|b1 (per-core)
    din("wsq", [D, 128])                # diag(ln1w) @ Wq slice (per-core)
    din("wsk", [D, 128])
    din("wsv", [D, 128])
    din("qkvf", [2, 3 * 128])           # [ncolsum; wbias] per q|k|v
    din("wproj", [D, D])                # attn proj (shared)
    din("bprow", [1, D])
    din("w2p", [D, FC])                 # diag(ln2w) @ Wfc (shared)
    din("fcf", [2, FC])                 # [-colsum(w2p); Wfc^T ln2b + bfc]
    din("wm", [FC, D])                  # mlp proj (shared)
    din("bmrow", [1, D])
    din("masksD", [128, 8 * BLK])       # doubled causal masks per dc
    din("onesr", [1, BLK])              # ones row
    din("onesc", [128, 16])             # ones cols (v_nat augment)
    din("ones128", [128, 1])            # ones col (stats lhsT)
    din("identb", [128, 128])           # bf16 identity (PE transpose)
    din("hsel", [2, 128])               # [1..1 0..0; 0..0 1..1] halves
    io["out"] = nc.dram_tensor("out", [D, 2 * HB], F32,
                               kind="ExternalOutput").ap()
    if DBG:
        io["dbg_gst"] = nc.dram_tensor("dbg_gst", [2 * NB, BLK], BF16,
                                       kind="ExternalOutput").ap()
        io["dbg_qt"] = nc.dram_tensor("dbg_qt", [128, BLK], BF16,
                                      kind="ExternalOutput").ap()
        io["dbg_kt"] = nc.dram_tensor("dbg_kt", [128, BLK], BF16,
                                      kind="ExternalOutput").ap()
        io["dbg_vn"] = nc.dram_tensor("dbg_vn", [128, 2 * 130], BF16,
                                      kind="ExternalOutput").ap()
        io["dbg_a2a"] = nc.dram_tensor("dbg_a2a", [NC * 130, HB], BF16,
                                       kind="ExternalOutput").ap()
        io["dbg_x2"] = nc.dram_tensor("dbg_x2", [128, 2 * HB], BF16,
                                      kind="ExternalOutput").ap()
        io["dbg_sb"] = nc.dram_tensor("dbg_sb", [1, BLK], BF16,
                                      kind="ExternalOutput").ap()
        io["dbg_dens"] = nc.dram_tensor("dbg_dens", [32, HB], BF16,
                                        kind="ExternalOutput").ap()
        io["dbg_rden"] = nc.dram_tensor("dbg_rden", [32, HB], BF16,
                                        kind="ExternalOutput").ap()
        io["dbg_yn"] = nc.dram_tensor("dbg_yn", [128, HB], BF16,
                                      kind="ExternalOutput").ap()
        io["dbg_a2a1"] = nc.dram_tensor("dbg_a2a1", [NC * 130, HB], BF16,
                                        kind="ExternalOutput").ap()

    with tile.TileContext(nc) as tc:
        _emit(nc, tc, io)
    nc.compile()
    return nc


def _emit(nc, tc, io):
    cst = tc.alloc_tile_pool(name="cst", bufs=1)
    dram = tc.alloc_tile_pool(name="dram", bufs=1, space="DRAM")

    # ---------------- small shared constants ----------------
    ones128 = cst.tile([128, 1], BF16, tag="ones128")
    nc.sync.dma_start(ones128[:], io["ones128"][:])
    eps_t = cst.tile([1, 1], F32, tag="eps")
    nc.vector.memset(eps_t[:], EPS)

    # ---------------- constants / weights ----------------
    masksD = cst.tile([128, 8 * BLK], BF16, tag="masksD")
    nc.sync.dma_start(masksD[:], io["masksD"][:])
    qkvf = cst.tile([2, 3 * 128], BF16, tag="qkvf")
    nc.sync.dma_start(qkvf[:], io["qkvf"][:])
    fcf = cst.tile([2, FC], BF16, tag="fcf")
    nc.sync.dma_start(fcf[:], io["fcf"][:])
    bprow = cst.tile([1, D], BF16, tag="bprow")
    nc.sync.dma_start(bprow[:], io["bprow"][:])
    bmrow = cst.tile([1, D], BF16, tag="bmrow")
    nc.sync.dma_start(bmrow[:], io["bmrow"][:])
    onesr = cst.tile([1, BLK], BF16, tag="onesr")
    nc.sync.dma_start(onesr[:], io["onesr"][:])
    identb = cst.tile([128, 128], BF16, tag="identb")
    nc.sync.dma_start(identb[:], io["identb"][:])
    hsel = cst.tile([2, 128], BF16, tag="hsel")
    nc.sync.dma_start(hsel[:], io["hsel"][:])

    ws = {}
    for nm in ("q", "k", "v"):
        wt = cst.tile([128, D], BF16, tag=f"ws{nm}")
        wd = io["ws" + nm]
        for c in range(DC):
            nc.sync.dma_start(wt[:, 128 * c:128 * (c + 1)],
                              wd[128 * c:128 * (c + 1), :])
        ws[nm] = wt

    wpp = tc.alloc_tile_pool(name="wpp", bufs=8)
    wp_sb = []
    for c in range(DC):
        wt = wpp.tile([128, D], BF16, tag="wp")
        nc.sync.dma_start(wt[:], io["wproj"][128 * c:128 * (c + 1), :])
        wp_sb.append(wt)

    a2a_in = [dram.tile([NC * 130, HB], BF16, tag=f"a2a_in{b}",
                        name=f"a2a_in{b}") for b in range(B)]
    a2a_out = [dram.tile([NC * 130, HB], BF16, tag=f"a2a_out{b}",
                         name=f"a2a_out{b}") for b in range(B)]

    denp = tc.alloc_tile_pool(name="denp", bufs=1)
    dens = [denp.tile([16, HB], BF16, tag=f"dens{b}", name=f"dens{b}")
            for b in range(B)]
    rden = [denp.tile([16, HB], BF16, tag=f"rden{b}", name=f"rden{b}")
            for b in range(B)]
    rdp = {}
    for b in range(B):
        for c in range(DC):
            rdp[(b, c)] = denp.tile([2, HB], BF16, tag=f"rdp{b}_{c}",
                                    name=f"rdp{b}_{c}")

    def den_chain(b):
        nc.sync.dma_start(
            dens[b][:],
            a2a_out[b][:].rearrange("(c r) q -> c r q", r=130)[:, 128:130, :])
        with nc.allow_low_precision(reason="softmax denom recip bf16"):
            nc.vector.reciprocal(rden[b][:], dens[b][:])
        for c in range(DC):
            nc.sync.dma_start(rdp[(b, c)][:], rden[b][2 * c:2 * c + 2, :])

    big = tc.alloc_tile_pool(name="big", bufs=1)
    qt = big.tile([128, TOK], BF16, tag="qt")
    kt = big.tile([128, TOK], BF16, tag="kt")
    # v_nat[b] per 128-token chunk tc: [64 h0 | ones | 64 h1 | ones]
    v_nat = [big.tile([128, 16 * 130], BF16, tag=f"vnat{b}",
                      name=f"vnat{b}") for b in range(B)]
    on3 = io["onesc"].rearrange("p (t o) -> p t o", o=1)
    for b in range(B):
        vn3 = v_nat[b][:].rearrange("p (t s) -> p t s", s=130)
        nc.sync.dma_start(vn3[:, :, 64:65], on3[:, 0:16, :])
        nc.sync.dma_start(vn3[:, :, 129:130], on3[:, 0:16, :])

    # per-block LN1 helper tiles (partition-0-aligned)
    sxp = tc.alloc_tile_pool(name="sxp", bufs=1)
    vtp = tc.alloc_tile_pool(name="vtp", bufs=1)
    vt = vtp.tile([128, TOK], BF16, tag="vt")
    srow, strhs, sbc = [], [], []
    for j in range(NB):
        strhs.append(sxp.tile([2, BLK], BF16, tag=f"strhs{j}",
                              name=f"strhs{j}"))
        srow.append(sxp.tile([1, BLK], BF16, tag=f"srow{j}",
                             name=f"srow{j}"))
        sbc.append(sxp.tile([128, BLK], BF16, tag=f"sbc{j}",
                            name=f"sbc{j}"))

    # =================== Phase 1: LN1-folded QKV ===================
    # Per block: psum = W'^T x^T  (+ fold MM: ncs*mu + wb*sd), then
    # qt/kt/vt = psum * s_bc.  Fold MMs are emitted one block late so
    # the PE never waits on the AllGather at the queue head.
    NMS = ("q", "k", "v")
    with tc.tile_pool(name="xtp", bufs=32) as xtp, \
            tc.tile_pool(name="sqp", bufs=16) as sqp, \
            tc.tile_pool(name="st1", bufs=3) as st1, \
            tc.tile_pool(name="ps0", bufs=1, space="PSUM") as ps0, \
            tc.tile_pool(name="ps1", bufs=2, space="PSUM") as ps1:
        xts = {}
        psb = {}

        def load_block(j):
            t0 = BLK * j
            xts[j] = []
            for c in range(DC):
                xt_c = xtp.tile([128, BLK], BF16, tag="xt")
                nc.sync.dma_start(xt_c[:],
                                  io["xT"][128 * c:128 * (c + 1),
                                           t0:t0 + BLK])
                xts[j].append(xt_c)

        def stats_block(j):
            sqs = []
            for c in range(DC):
                sq = sqp.tile([128, BLK], BF16, tag="sq")
                nc.scalar.square(sq[:], xts[j][c][:])
                sqs.append(sq)
            mu_ps = ps0.tile([1, BLK], F32, tag="mu_ps")
            sq_ps = ps0.tile([1, BLK], F32, tag="sq_ps")
            for c in range(DC):
                nc.tensor.matmul(mu_ps[:], ones128[:], xts[j][c][:],
                                 start=(c == 0), stop=(c == DC - 1))
            for c in range(DC):
                nc.tensor.matmul(sq_ps[:], ones128[:], sqs[c][:],
                                 start=(c == 0), stop=(c == DC - 1))
            mu_row = st1.tile([1, BLK], F32, tag="mu_row")
            nc.vector.tensor_scalar_mul(mu_row[:], mu_ps[:], 1.0 / D)
            var_row = st1.tile([1, BLK], F32, tag="var_row")
            nc.vector.tensor_mul(var_row[:], mu_row[:], mu_row[:])
            nc.vector.scalar_tensor_tensor(var_row[:], sq_ps[:], 1.0 / D,
                                           var_row[:], op0=AL.mult,
                                           op1=AL.subtract)
            sdr = st1.tile([1, BLK], BF16, tag="sdr")
            nc.scalar.activation(sdr[:], var_row[:], AF.Sqrt, bias=eps_t[:])
            nc.vector.tensor_copy(strhs[j][0:1, :], mu_row[:])
            nc.sync.dma_start(strhs[j][1:2, :], sdr[:])
            lnr = st1.tile([1, BLK], F32, tag="lnr")
            nc.scalar.activation(lnr[:], sdr[:], AF.Ln)
            nc.scalar.activation(srow[j][:], lnr[:], AF.Exp, scale=-1.0)
            nc.gpsimd.partition_broadcast(sbc[j][:], srow[j][:])

        def qkv_mms(j):
            psb[j] = {}
            for nm in NMS:
                o_ps = ps1.tile([128, BLK], F32, tag=f"ps_{nm}",
                                name=f"ps_{nm}_{j}")
                for c in range(DC):
                    nc.tensor.matmul(o_ps[:],
                                     ws[nm][:, 128 * c:128 * (c + 1)],
                                     xts[j][c][:],
                                     start=(c == 0), stop=False)
                psb[j][nm] = o_ps

        def fold_and_scale(j):
            t0 = BLK * j
            for idx, nm in enumerate(NMS):
                o_ps = psb[j][nm]
                nc.tensor.matmul(o_ps[:],
                                 qkvf[:, 128 * idx:128 * (idx + 1)],
                                 strhs[j][:],
                                 start=False, stop=True)
                dst = (qt if nm == "q" else kt if nm == "k" else vt)
                nc.vector.tensor_mul(dst[:, t0:t0 + BLK], o_ps[:],
                                     sbc[j][:])
            del psb[j]
            del xts[j]

        load_block(0)
        load_block(1)
        stats_block(0)
        qkv_mms(0)
        load_block(2)
        stats_block(1)
        qkv_mms(1)
        for j in range(2, NB):
            if j + 1 < NB:
                load_block(j + 1)
            fold_and_scale(j - 2)
            stats_block(j)
            qkv_mms(j)
        fold_and_scale(NB - 2)
        fold_and_scale(NB - 1)

    # =================== Phase 1b: V^T -> V_nat (both batches) ========
    # Done before the attention PSUM pools claim all 8 banks.
    with tc.tile_pool(name="pstp", bufs=2, space="PSUM") as pstp:
        for gtc in range(32):
            b, tci = gtc // 16, gtc % 16
            tp = pstp.tile([128, 1024], BF16, tag="tp")
            nc.tensor.transpose(tp[:, 0:128],
                                vt[:, 128 * gtc:128 * (gtc + 1)],
                                identb[:])
            vn4 = v_nat[b][:].rearrange("p (t g s) -> p t g s",
                                        g=2, s=65)
            nc.vector.tensor_copy(
                vn4[:, tci, :, 0:64],
                tp[:, 0:128].rearrange("p (g s) -> p g s", s=64))
    vtp.release()

    # =================== Phase 2: attention ===================
    # Work items (b, jb, c): k-chunk c of q-block jb of batch b, both
    # heads at once.  Scores run LOOKAHEAD items ahead of the AV
    # accumulations so the PE does not stall on ScalarE's exp.
    LOOKAHEAD = 2

    att = tc.alloc_tile_pool(name="att", bufs=4)
    yup = tc.alloc_tile_pool(name="yup", bufs=2)
    psS = tc.alloc_tile_pool(name="psS", bufs=2, space="PSUM")
    psA = tc.alloc_tile_pool(name="psA", bufs=2, space="PSUM")

    av_cur = {}
    pend = []

    def start_scores(it):
        b, jb, c = it
        q0 = 2048 * b + 512 * jb
        k0 = 2048 * b + 128 * c
        s_ps = psS.tile([128, 2 * BLK], F32, tag="s", name="s")
        for h in range(2):
            hr0 = 64 * h
            nc.tensor.matmul(s_ps[:, BLK * h:BLK * (h + 1)],
                             kt[hr0:hr0 + 64, k0:k0 + 128],
                             qt[hr0:hr0 + 64, q0:q0 + BLK],
                             start=True, stop=True)
        e = att.tile([128, 2 * BLK], BF16, tag="e", name="e")
        dc_ = c - 4 * jb
        if dc_ < 0:
            nc.scalar.activation(e[:], s_ps[:], AF.Exp, scale=0.125)
        else:
            off = 128 * dc_
            e3 = e[:].rearrange("p (g q) -> p g q", q=BLK)
            s3 = s_ps[:].rearrange("p (g q) -> p g q", q=BLK)
            m3 = masksD[:, 1024 * dc_:1024 * (dc_ + 1)].rearrange(
                "p (g q) -> p g q", q=BLK)
            if off:
                nc.vector.memset(e3[:, :, 0:off], 0.0)
            nc.scalar.activation(e3[:, :, off:BLK], s3[:, :, off:BLK],
                                 AF.Exp, scale=0.125)
            nc.vector.tensor_mul(e3[:, :, off:BLK], e3[:, :, off:BLK],
                                 m3[:, :, off:BLK])
        return e

    def flush_one():
        (b, jb, c), e = pend.pop(0)
        nk = 4 * jb + 4
        if c == 0:
            av_cur[(b, jb)] = psA.tile([65, 2 * BLK], F32, tag="av",
                                       name="av")
        av = av_cur[(b, jb)]
        for h in range(2):
            nc.tensor.matmul(
                av[:, BLK * h:BLK * (h + 1)],
                v_nat[b][:, 130 * c + 65 * h:130 * c + 65 * (h + 1)],
                e[:, BLK * h:BLK * (h + 1)],
                start=(c == 0), stop=(c == nk - 1))
        if c == nk - 1:
            av = av_cur.pop((b, jb))
            yu = yup.tile([65, 2 * BLK], BF16, tag="yu", name="yu")
            nc.vector.tensor_copy(yu[:], av[:])
            yu3 = yu[:].rearrange("p (g q) -> p g q", q=BLK)
            for half in range(2):
                rk = 2 * jb + half
                dsty = a2a_in[b][130 * rk:130 * rk + 128, :].rearrange(
                    "(g p) q -> p g q", g=2)
                nc.sync.dma_start(dsty,
                                  yu3[0:64, :, HB * half:HB * (half + 1)])
                dstd = a2a_in[b][130 * rk + 128:130 * rk + 130, :]
                nc.sync.dma_start(dstd.rearrange("(g o) q -> o g q", o=1),
                                  yu3[64:65, :, HB * half:HB * (half + 1)])

    def fire_a2a(b):
        nc.gpsimd.collective_compute(
            "AllToAll", AL.bypass, replica_groups=[list(range(NC))],
            ins=[a2a_in[b].opt()], outs=[a2a_out[b].opt()])

    items0 = [(0, jb, c) for jb in range(4) for c in range(4 * jb + 4)]
    items1 = [(1, jb, c) for jb in range(4) for c in range(4 * jb + 4)]

    for it in items0:
        pend.append((it, start_scores(it)))
        if len(pend) > LOOKAHEAD:
            flush_one()
    while pend:
        flush_one()
    fire_a2a(0)
    den_chain(0)
    for it in items1:
        pend.append((it, start_scores(it)))
        if len(pend) > LOOKAHEAD:
            flush_one()
    while pend:
        flush_one()
    fire_a2a(1)
    den_chain(1)

    if DBG:
        nc.sync.dma_start(io["dbg_qt"][:], qt[:, 0:BLK])
        nc.sync.dma_start(io["dbg_kt"][:], kt[:, 0:BLK])
        nc.sync.dma_start(io["dbg_vn"][:], v_nat[0][:, 0:260])
        nc.sync.dma_start(io["dbg_sb"][:], srow[0][:])
    psA.release()
    psS.release()
    yup.release()
    att.release()
    sxp.release()
    big.release()

    if DBG:
        nc.sync.dma_start(io["dbg_a2a"][:], a2a_out[0][:])
        nc.sync.dma_start(io["dbg_a2a1"][:], a2a_out[1][:])

    # Late weight streams (emitted after attention so their DMAs don't
    # starve the latency-critical early loads; pools sized to prefetch).
    w2pp = tc.alloc_tile_pool(name="w2pp", bufs=16)
    wmp = tc.alloc_tile_pool(name="wmp", bufs=6)

    # =================== Phase 4: y -> proj -> x2 =====================
    ph4 = tc.alloc_tile_pool(name="ph4", bufs=1)

    yn = [[None] * DC for _ in range(B)]
    with tc.tile_pool(name="ynp", bufs=1) as ynp:
      with tc.tile_pool(name="psrb", bufs=2, space="PSUM") as psrb:
        for b in range(B):
            for c in range(DC):
                ysb = ynp.tile([128, HB], BF16, tag=f"ysb{b}_{c}",
                               name=f"ysb{b}_{c}")
                nc.sync.dma_start(ysb[:], a2a_out[b][130 * c:130 * c + 128, :])
                rb = psrb.tile([128, BLK], F32, tag="rb")
                nc.tensor.matmul(rb[:, 0:HB], hsel[:], rdp[(b, c)][:],
                                 start=True, stop=True)
                yt = ynp.tile([128, HB], BF16, tag=f"yn{b}_{c}",
                              name=f"yn{b}_{c}")
                nc.vector.tensor_mul(yt[:], ysb[:], rb[:, 0:HB])
                if DBG and b == 0 and c == 0:
                    nc.sync.dma_start(io["dbg_yn"][:], yt[:])
                yn[b][c] = yt

      if True:
        xo_sb = []
        for m in range(DC):
            xm = ph4.tile([128, 2 * HB], BF16, tag=f"xo{m}", name=f"xo{m}")
            nc.sync.dma_start(xm[:], io["xo"][128 * m:128 * (m + 1), :])
            xo_sb.append(xm)

        # z^T = Wp^T y^T + bproj, x2 = z^T + x^T   (all [128, 512] tiles)
        # One batch-wave at a time (8 PSUM banks per wave); batch-0 wave
        # runs while a2a(1) is still in flight.
        x2 = [ph4.tile([128, 2 * HB], BF16, tag=f"x2_{m}", name=f"x2_{m}")
              for m in range(DC)]
        x2sq = []
        with tc.tile_pool(name="ps4", bufs=1, space="PSUM") as ps4:
            for b in range(B):
                zps = []
                for m in range(DC):
                    # full-bank tile (one generation per batch wave)
                    zp = ps4.tile([128, BLK], F32, tag=f"zp{m}",
                                  name=f"zp{b}_{m}")
                    for c in range(DC):
                        nc.tensor.matmul(zp[:, 0:HB],
                                         wp_sb[c][:, 128 * m:128 * (m + 1)],
                                         yn[b][c][:],
                                         start=(c == 0), stop=False)
                    nc.tensor.matmul(zp[:, 0:HB],
                                     bprow[0:1, 128 * m:128 * (m + 1)],
                                     onesr[0:1, 0:HB], start=False, stop=True)
                    zps.append(zp)
                for m in range(DC):
                    nc.vector.tensor_add(x2[m][:, HB * b:HB * (b + 1)],
                                         zps[m][:, 0:HB],
                                         xo_sb[m][:, HB * b:HB * (b + 1)])


    if DBG:
        nc.sync.dma_start(io["dbg_x2"][:], x2[0][:])
        nc.sync.dma_start(io["dbg_dens"][0:16, :], dens[0][:])
        nc.sync.dma_start(io["dbg_dens"][16:32, :], dens[1][:])
        nc.sync.dma_start(io["dbg_rden"][0:16, :], rden[0][:])
        nc.sync.dma_start(io["dbg_rden"][16:32, :], rden[1][:])

    # LN2 stats over x2 (own 512 tokens)
    st2p = tc.alloc_tile_pool(name="st2p", bufs=1)
    stats2 = st2p.tile([2, BLK], BF16, tag="stats2")
    sdrow0 = st2p.tile([1, BLK], BF16, tag="sdrow0")
    sbc2 = st2p.tile([128, BLK], BF16, tag="sbc2")
    with tc.tile_pool(name="ps2s", bufs=2, space="PSUM") as ps2s, \
            tc.tile_pool(name="st2t", bufs=1) as st2t:
        for m in range(DC):
            sq = st2t.tile([128, 2 * HB], BF16, tag=f"x2sq{m}",
                           name=f"x2sq{m}")
            nc.scalar.square(sq[:], x2[m][:])
            x2sq.append(sq)
        mu_ps = ps2s.tile([1, BLK], F32, tag="mu2")
        sq_ps = ps2s.tile([1, BLK], F32, tag="sq2")
        for c in range(DC):
            nc.tensor.matmul(mu_ps[:], ones128[:], x2[c][:],
                             start=(c == 0), stop=(c == DC - 1))
        for c in range(DC):
            nc.tensor.matmul(sq_ps[:], ones128[:], x2sq[c][:],
                             start=(c == 0), stop=(c == DC - 1))
        mu_row = st2t.tile([1, BLK], F32, tag="mu_row2")
        nc.vector.tensor_scalar_mul(mu_row[:], mu_ps[:], 1.0 / D)
        var_row = st2t.tile([1, BLK], F32, tag="var_row2")
        nc.vector.tensor_mul(var_row[:], mu_row[:], mu_row[:])
        nc.vector.scalar_tensor_tensor(var_row[:], sq_ps[:], 1.0 / D,
                                       var_row[:], op0=AL.mult,
                                       op1=AL.subtract)
        nc.vector.tensor_copy(stats2[0:1, :], mu_row[:])
        nc.scalar.activation(sdrow0[:], var_row[:], AF.Sqrt, bias=eps_t[:])
        nc.sync.dma_start(stats2[1:2, :], sdrow0[:])
        lnr = st2t.tile([1, BLK], F32, tag="lnr2")
        nc.scalar.activation(lnr[:], sdrow0[:], AF.Ln)
        srow2 = st2t.tile([1, BLK], BF16, tag="srow2")
        nc.scalar.activation(srow2[:], lnr[:], AF.Exp, scale=-1.0)
        nc.gpsimd.partition_broadcast(sbc2[:], srow2[:])

    # =================== Phase 5: MLP (LN2 folded) ====================
    # u = W2'^T x2^T - colsum*mu + c*sd ; r = relu(u) ; out = x2 + s*(
    #   Wm^T r + bmlp*sd )
    rpool = tc.alloc_tile_pool(name="rpool", bufs=1)
    rr = []
    with tc.tile_pool(name="ps5a", bufs=3, space="PSUM") as ps5a:
        for g in range(NFC // 4):
            w2g = []
            for c in range(DC):
                wt = w2pp.tile([128, BLK], BF16, tag="w2g")
                nc.sync.dma_start(
                    wt[:], io["w2p"][128 * c:128 * (c + 1),
                                     BLK * g:BLK * (g + 1)])
                w2g.append(wt)
            for fi in range(4):
                f = 4 * g + fi
                ps = ps5a.tile([128, BLK], F32, tag="psfc")
                for c in range(DC):
                    nc.tensor.matmul(ps[:],
                                     w2g[c][:, 128 * fi:128 * (fi + 1)],
                                     x2[c][:],
                                     start=(c == 0), stop=False)
                nc.tensor.matmul(ps[:], fcf[:, 128 * f:128 * (f + 1)],
                                 stats2[:], start=False, stop=True)
                rt = rpool.tile([128, BLK], BF16, tag=f"r{f}", name=f"r{f}")
                nc.scalar.activation(rt[:], ps[:], AF.Relu)
                rr.append(rt)

    with tc.tile_pool(name="ps5b", bufs=1, space="PSUM") as ps5b, \
            tc.tile_pool(name="fin", bufs=4) as fin:
        pvs = [ps5b.tile([128, BLK], F32, tag=f"pv{m}", name=f"pv{m}")
               for m in range(DC)]
        for f in range(NFC):
            wt = wmp.tile([128, D], BF16, tag="wm")
            nc.sync.dma_start(wt[:], io["wm"][128 * f:128 * (f + 1), :])
            for m in range(DC):
                nc.tensor.matmul(pvs[m][:],
                                 wt[:, 128 * m:128 * (m + 1)],
                                 rr[f][:],
                                 start=(f == 0), stop=False)
        for m in range(DC):
            nc.tensor.matmul(pvs[m][:], bmrow[0:1, 128 * m:128 * (m + 1)],
                             sdrow0[:], start=False, stop=True)
            tmp = fin.tile([128, BLK], BF16, tag="tmp")
            nc.vector.tensor_mul(tmp[:], pvs[m][:], sbc2[:])
            fo = fin.tile([128, BLK], F32, tag="fo")
            nc.vector.tensor_add(fo[:], tmp[:], x2[m][:])
            nc.sync.dma_start(io["out"][128 * m:128 * (m + 1), :], fo[:])

    rpool.release()
    st2p.release()
    ph4.release()
    wmp.release()
    w2pp.release()
    denp.release()
    wpp.release()
    dram.release()
    cst.release()


_NC_CACHE = None


def _get_nc():
    global _NC_CACHE
    if _NC_CACHE is None:
        _NC_CACHE = build()
    return _NC_CACHE


def _make_masksD():
    kk = np.arange(128)[:, None]
    qq = np.arange(BLK)[None, :]
    m = np.zeros((128, 8 * BLK), np.float32)
    for dc in range(4):
        blkm = (128 * dc + kk <= qq).astype(np.float32)
        m[:, 1024 * dc:1024 * dc + 512] = blkm
        m[:, 1024 * dc + 512:1024 * (dc + 1)] = blkm
    return m


def prepare_in_maps(inputs):
    bf = ml_dtypes.bfloat16
    x = np.asarray(inputs["x"], np.float32)
    ln1w = np.asarray(inputs["ln1_w"], np.float32)
    ln1b = np.asarray(inputs["ln1_b"], np.float32)
    ln2w = np.asarray(inputs["ln2_w"], np.float32)
    ln2b = np.asarray(inputs["ln2_b"], np.float32)
    w_attn = np.asarray(inputs["w_attn"], np.float32)
    b_attn = np.asarray(inputs["b_attn"], np.float32)
    wproj = np.asarray(inputs["w_attn_proj"], np.float32)
    bproj = np.asarray(inputs["b_attn_proj"], np.float32)
    wfc = np.asarray(inputs["w_fc"], np.float32)
    bfc = np.asarray(inputs["b_fc"], np.float32)
    wmlp = np.asarray(inputs["w_mlp_proj"], np.float32)
    bmlp = np.asarray(inputs["b_mlp_proj"], np.float32)

    xf = np.ascontiguousarray(x.reshape(TOK, D))
    xT = np.ascontiguousarray(xf.T).astype(bf)

    w2p = np.ascontiguousarray(ln2w[:, None] * wfc)
    fcf = np.ascontiguousarray(np.stack(
        [-w2p.sum(axis=0), wfc.T @ ln2b + bfc], axis=0))

    shared = {
        "xT": xT,
        "wproj": np.ascontiguousarray(wproj).astype(bf),
        "bprow": bproj.reshape(1, D).astype(bf),
        "w2p": w2p.astype(bf),
        "fcf": fcf.astype(bf),
        "wm": np.ascontiguousarray(wmlp).astype(bf),
        "bmrow": bmlp.reshape(1, D).astype(bf),
        "masksD": _make_masksD().astype(bf),
        "onesr": np.ones((1, BLK), bf),
        "onesc": np.ones((128, 16), bf),
        "ones128": np.ones((128, 1), bf),
        "identb": np.eye(128, dtype=np.float32).astype(bf),
        "hsel": np.stack([np.r_[np.ones(64), np.zeros(64)],
                          np.r_[np.zeros(64), np.ones(64)]]).astype(bf),
    }
    in_maps = []
    for i in range(NC):
        f0 = 128 * i
        m = dict(shared)
        # own token cols: batch 0 then batch 1
        m["xo"] = np.ascontiguousarray(np.concatenate(
            [xT[:, HB * i:HB * (i + 1)],
             xT[:, T + HB * i:T + HB * (i + 1)]], axis=1))
        qf = []
        for p, nm in enumerate(("q", "k", "v")):
            wsl = w_attn[:, D * p + f0:D * p + f0 + 128]
            wsc = ln1w[:, None] * wsl
            m["ws" + nm] = np.ascontiguousarray(wsc).astype(bf)
            ncs = -wsc.sum(axis=0)
            wb = wsl.T @ ln1b + b_attn[D * p + f0:D * p + f0 + 128]
            qf.append(np.stack([ncs, wb], axis=0))
        m["qkvf"] = np.ascontiguousarray(
            np.concatenate(qf, axis=1)).astype(bf)
        in_maps.append(m)
    return in_maps


def run(inputs, trace=False):
    nc = _get_nc()
    in_maps = prepare_in_maps(inputs)
    res = run_bass_kernel_spmd(nc, in_maps, list(range(NC)), trace=trace)
    full = np.empty((B, T, D), np.float32)
    for i in range(NC):
        blk = res.results[i]["out"]          # [D, 512] (b0 | b1)
        full[0, HB * i:HB * (i + 1)] = blk[:, 0:HB].T
        full[1, HB * i:HB * (i + 1)] = blk[:, HB:2 * HB].T
    return full, res


def kernel(**inputs):
    full, _ = run(inputs, trace=False)
    return full
